# revision 28
# baseline (speedup 1.0000x reference)
"""Trainium2 Bass kernel for causal multi-head attention with RoPE.

Problem: B=1, S=4096, D=1024, H=16 heads of HD=64.
  q/k/v = x @ w{q,k,v}.T ; rope(q), rope(k); scores = q k^T/sqrt(HD) + mask;
  out = softmax(scores) @ v ; y = out @ wo.T

Sharding: tensor-parallel over heads. 8 cores x 2 heads each.  Each core
computes its 2 heads' q/k/v projections (column-split weights), full
attention for those heads over all 4096 positions, and a partial output
projection (row-split wo).  The host sums the 8 partial [S, D] outputs.

v5 structure (ACT-engine exp is the roofline; fp8 DoubleRow halves the
score matmuls; probs/v/pv stay bf16 for accuracy):
  - q/k (after rope) are stored fp8e4 with host-folded scales in a
    [32, 4S] layout (4 k-tiles: headA-lo, headA-hi, headB-lo, headB-hi);
    score matmuls run in MatmulPerfMode.DoubleRow at 0.5 cycles/row,
    contracting hd=64 as 2 k-tiles of 32.  exp() recovers the scale via
    the activation scale operand; the mask is pre-scaled on the host.
  - The rope output is written fp8 into a 2-block staging tile and
    flushed to the [32, 4S] layout by partition-remap DMAs each block
    pair; attn(J) is therefore emitted after phase1(J+1).
  - probs (bf16) / v (bf16 with ones column) / pv matmuls are the v3
    scheme: two heads share one [128, 1024] psum score tile, one exp
    covers both, pv accumulates [65, 512] per head with the denominator
    in row 64.
  - Engine rebalance: psum->sbuf casts, rope cos-mul, mask adds and
    prob zero-fills run on Pool (gpsimd) instead of DVE; the softmax
    reciprocal runs directly on partition 64 and is broadcast by a K=1
    fp32 PE matmul (no partition-move DMAs in the normalize chain);
    head B is normalized first so its extra SBUF shift DMA overlaps
    head A's chain.
  - Startup DMAs are spread across the gpsimd + sync + scalar hardware
    queues with the first projection block's chunks first.
  - y partials are written bf16, one consolidated DMA per 512-row
    block; outproj slots interleave into the next J's score chunks.
"""

import os
import sys

import numpy as np

sys.path.insert(0, "/opt/trn_rl_repo")

S = 4096
D = 1024
H = 16
HD = 64
NCORES = 8
HPC = H // NCORES  # 2 heads per core
EC = HPC * HD  # 128 head-dim columns per core
NEG_THRESH = -1e8  # blocks entirely <= this are "fully masked"

_PROGRAM_CACHE = {}
SCORES_MODE = os.environ.get("V5_SCORES", "bf16")  # fp8 | bf16
NORM_MODE = os.environ.get("V5_NORM", "v3")  # pe | v3 | direct
POOL_MODE = os.environ.get("V5_POOL", "off")  # on | off (gpsimd tensor ops)


def classify_mask(maskT, s):
    """Classify 128x128 blocks of mask.T: 0=zero, 1=neginf, 2=general."""
    nb = s // 128
    cls = np.zeros((nb, nb), np.int8)
    for i in range(nb):
        for j in range(nb):
            blk = maskT[128 * i : 128 * (i + 1), 128 * j : 128 * (j + 1)]
            if np.all(blk == 0.0):
                cls[i, j] = 0
            elif np.all(blk <= NEG_THRESH):
                cls[i, j] = 1
            else:
                cls[i, j] = 2
    return cls


def build_program(s, cls, n_gen, neg_bias, sc_scale):
    """Build the SPMD Bass/Tile program for one core (same for all cores)."""
    from contextlib import ExitStack

    import concourse.bass as bass
    import concourse.tile as tile
    from concourse import bacc, mybir

    f32 = mybir.dt.float32
    bf16 = mybir.dt.bfloat16
    fp8 = mybir.dt.float8e4
    Exp = mybir.ActivationFunctionType.Exp
    DR = mybir.MatmulPerfMode.DoubleRow

    nb = s // 128  # sk chunks
    NJ = s // 512  # sq blocks
    nd = D // 128  # contraction chunks for projections

    nc = bacc.Bacc(
        "TRN2", target_bir_lowering=False, debug=False, num_devices=NCORES
    )

    # xP / csP / yP are host-shuffled so every per-sb DMA is a plain
    # contiguous [128, N] block.
    xP = nc.dram_tensor("xP", [128, (D // 128) * s], bf16, kind="ExternalInput").ap()
    csP = nc.dram_tensor("csP", [128, 2 * s], bf16, kind="ExternalInput").ap()
    wqT = nc.dram_tensor("wqT", [128, D], bf16, kind="ExternalInput").ap()
    wkT = nc.dram_tensor("wkT", [128, D], bf16, kind="ExternalInput").ap()
    wvT = nc.dram_tensor("wvT", [128, D], bf16, kind="ExternalInput").ap()
    woT = nc.dram_tensor("woT", [128, D], bf16, kind="ExternalInput").ap()
    P128 = nc.dram_tensor("P128", [128, 128], bf16, kind="ExternalInput").ap()
    I128 = nc.dram_tensor("I128", [128, 128], bf16, kind="ExternalInput").ap()
    maskg = nc.dram_tensor(
        "maskg", [128, 128 * max(n_gen, 1)], f32, kind="ExternalInput"
    ).ap()
    ones2 = nc.dram_tensor("ones2", [128, 130], bf16, kind="ExternalInput").ap()
    ones64 = nc.dram_tensor("ones64", [128, 64], f32, kind="ExternalInput").ap()
    biasC = nc.dram_tensor("biasC", [128, 1], f32, kind="ExternalInput").ap()
    yP = nc.dram_tensor("yP", [128, (s // 512) * 4096], bf16, kind="ExternalOutput").ap()

    with tile.TileContext(nc) as tc, ExitStack() as ctx:
        consts = ctx.enter_context(tc.tile_pool(name="consts", bufs=1))
        persist = ctx.enter_context(tc.tile_pool(name="persist", bufs=1))
        xt_pool = ctx.enter_context(tc.tile_pool(name="xt", bufs=2))
        wk_pool = ctx.enter_context(tc.tile_pool(name="work", bufs=6))
        probs_pool = ctx.enter_context(tc.tile_pool(name="probs", bufs=6))
        attn_pool = ctx.enter_context(tc.tile_pool(name="attn", bufs=3))
        yo_pool = ctx.enter_context(tc.tile_pool(name="yo", bufs=2))
        psum = ctx.enter_context(tc.tile_pool(name="psum", bufs=2, space="PSUM"))

        # ---- constants, spread across the three DMA queues.  gpsimd gets
        # everything phase1(0)'s q-pass touches (wq + x dc 0..3); sync gets
        # x dc 4..7; scalar gets the rest of phase1(0)'s needs.
        c_wq = consts.tile([128, D], bf16)
        c_cs = consts.tile([128, 2 * s], bf16)
        xts0 = xt_pool.tile([128, nd * 512], bf16)
        for dc in range(nd):
            if dc < 4:
                nc.gpsimd.dma_start(
                    c_wq[:, 128 * dc : 128 * (dc + 1)],
                    wqT[:, 128 * dc : 128 * (dc + 1)],
                )
                nc.gpsimd.dma_start(
                    xts0[:, 512 * dc : 512 * (dc + 1)],
                    xP[:, 512 * dc : 512 * (dc + 1)],
                )
            else:
                nc.sync.dma_start(
                    xts0[:, 512 * dc : 512 * (dc + 1)],
                    xP[:, 512 * dc : 512 * (dc + 1)],
                )
        nc.gpsimd.dma_start(c_wq[:, 512:1024], wqT[:, 512:1024])
        nc.sync.dma_start(c_cs[:, 0:1024], csP[:, 0:1024])
        c_P = consts.tile([128, 128], bf16)
        nc.sync.dma_start(c_P[:], P128[:])
        c_wk = consts.tile([128, D], bf16)
        nc.sync.dma_start(c_wk[:], wkT[:])
        c_wv = consts.tile([128, D], bf16)
        nc.sync.dma_start(c_wv[:], wvT[:])
        c_I = consts.tile([128, 128], bf16)
        nc.sync.dma_start(c_I[:], I128[:])
        c_one2 = consts.tile([128, 130], bf16)  # cols 0:2 ones, 2:130 zeros
        nc.gpsimd.dma_start(c_one2[:], ones2[:])
        c_ones64 = consts.tile([128, 64], f32)
        nc.gpsimd.dma_start(c_ones64[:], ones64[:])
        c_bias = consts.tile([128, 1], f32)
        nc.gpsimd.dma_start(c_bias[:], biasC[:])
        # maskg (2MB) and wo are not needed until ~15us in; they go on the
        # scalar queue after the early blocks so they don't block x/cs.
        c_maskg = consts.tile([128, 128 * max(n_gen, 1)], f32)
        c_wo = consts.tile([128, D], bf16)

        def late_consts():
            nc.scalar.dma_start(c_maskg[:], maskg[:])
            nc.scalar.dma_start(c_wo[:], woT[:])

        # ---- persistent activations ----
        # qf8/kf8: [32, 4S] fp8.  col t*S+c holds, for partition p:
        #   t=0: head A rope'd dim p      at position c
        #   t=1: head A rope'd dim 32+p
        #   t=2: head B rope'd dim p
        #   t=3: head B rope'd dim 32+p
        v_sb = persist.tile([128, nb * 130], bf16)  # per sk-chunk: [A 64|1|B 64|1]
        if SCORES_MODE == "fp8":
            qf8 = persist.tile([32, 4 * s], fp8)
            kf8 = persist.tile([32, 4 * s], fp8)
            qq = qf8[:].rearrange("p (t c) -> p t c", c=s)
            kk = kf8[:].rearrange("p (t c) -> p t c", c=s)
        else:
            qT2 = persist.tile([128, s], bf16)
            kT2 = persist.tile([128, s], bf16)
        stg_cur = {}  # live fp8 staging tile per tensor ("q"/"k")

        # ones columns of v_aug (cols 64 and 129 of each 130-wide chunk)
        ones_view = v_sb[:].rearrange("p (c w) -> p c w", w=130)[:, :, 64:130:65]
        (nc.gpsimd if POOL_MODE == "on" else nc.vector).tensor_copy(
            ones_view, c_one2[:, None, 0:2].broadcast_to([128, nb, 2])
        )

        def phase1(sb):
            """Projections + rope + v-transpose for rows [512*sb, 512*sb+512).
            q/k land (via rope) fp8 in a 2-block staging tile, flushed to
            qf8/kf8 with partition-remap DMAs each block pair."""
            csl = slice(1024 * sb, 1024 * sb + 512)  # cos chunk in c_cs
            snl = slice(1024 * sb + 512, 1024 * (sb + 1))  # sin chunk
            if sb == 0:
                xts = xts0
            else:
                ceng = nc.scalar if sb < 2 else nc.sync
                ceng.dma_start(c_cs[:, 1024 * sb : 1024 * (sb + 1)],
                               csP[:, 1024 * sb : 1024 * (sb + 1)])
                xts = xt_pool.tile([128, nd * 512], bf16)
                nc.sync.dma_start(xts[:], xP[:, nd * 512 * sb : nd * 512 * (sb + 1)])
            so = 512 * (sb % 2)  # column offset in the 2-block staging tiles
            for cw, which in ((c_wq, "q"), (c_wk, "k")):
                ps = psum.tile([128, 512], f32, tag="proj", bufs=2)
                for dc in range(nd):
                    nc.tensor.matmul(
                        ps[:], cw[:, 128 * dc : 128 * (dc + 1)],
                        xts[:, 512 * dc : 512 * (dc + 1)],
                        start=dc == 0, stop=dc == nd - 1,
                    )
                raw = wk_pool.tile([128, 512], bf16, tag="rope")
                nc.vector.tensor_copy(raw[:], ps[:])
                psw = psum.tile([128, 512], f32, tag="proj", bufs=2)
                nc.tensor.matmul(psw[:], c_P[:], raw[:], start=True, stop=True)
                t1 = wk_pool.tile([128, 512], bf16, tag="rope")
                peng = nc.gpsimd if POOL_MODE == "on" else nc.vector
                peng.tensor_mul(t1[:], raw[:], c_cs[:, csl])
                t2 = wk_pool.tile([128, 512], bf16, tag="rope")
                nc.vector.tensor_mul(t2[:], psw[:], c_cs[:, snl])
                if SCORES_MODE != "fp8":
                    dstT = qT2 if which == "q" else kT2
                    nc.vector.tensor_add(
                        dstT[:, 512 * sb : 512 * (sb + 1)], t1[:], t2[:]
                    )
                    continue
                if sb % 2 == 0:
                    stg_cur[which] = wk_pool.tile(
                        [128, 1024], fp8, tag="stg" + which, bufs=2,
                        name="stg" + which,
                    )
                stg = stg_cur[which]
                nc.vector.tensor_add(stg[:, so : so + 512], t1[:], t2[:])
                # flush staged blocks to the [32, 4S] DoubleRow layout
                if sb % 2 == 1 or sb == NJ - 1:
                    w = so + 512
                    c0 = 512 * (sb - sb % 2)
                    dst = qq if which == "q" else kk
                    deng = nc.sync
                    for t in range(4):
                        deng.dma_start(
                            dst[:, t, c0 : c0 + w],
                            stg[32 * t : 32 * (t + 1), 0:w],
                        )
            # v pass: psum -> sbuf, then per-128 transpose into natural layout
            psv = psum.tile([128, 512], f32, tag="proj", bufs=2)
            for dc in range(nd):
                nc.tensor.matmul(
                    psv[:], c_wv[:, 128 * dc : 128 * (dc + 1)],
                    xts[:, 512 * dc : 512 * (dc + 1)],
                    start=dc == 0, stop=dc == nd - 1,
                )
            vtt = wk_pool.tile([128, 512], bf16, tag="vtt", bufs=2)
            nc.vector.tensor_copy(vtt[:], psv[:])
            for k4 in range(4):
                sc = 4 * sb + k4
                pst = psum.tile([128, 1024], bf16, tag="proj", bufs=2)
                nc.tensor.transpose(
                    pst[:, 0:128], vtt[:, 128 * k4 : 128 * (k4 + 1)], c_I[:]
                )
                vdst = v_sb[:].rearrange("p (c w) -> p c w", w=65)[
                    :, 2 * sc : 2 * sc + 2, 0:64
                ]
                psrc = pst[:, 0:128].rearrange("p (c w) -> p c w", w=64)
                nc.vector.tensor_copy(vdst, psrc)

        def attn(J, slots, tail=False):
            """Attention for q rows [512J, 512J+512); interleaves `slots`
            (previous J's output-projection emitters) between score chunks.
            Returns this J's outproj slots.  tail=True delays slot
            consumption further (the previous normalize has no phase1
            to hide behind)."""
            kept = [
                i
                for i in range(nb)
                if any(cls[i, 4 * J + u] != 1 for u in range(4))
            ]
            if not kept:
                while slots:
                    slots.pop(0)()
                return []
            pvA = psum.tile([128, 512], f32, tag="pv", bufs=2)
            pvB = psum.tile([128, 512], f32, tag="pv", bufs=2)
            pvq = []  # chunks whose pv matmuls are not yet emitted

            def emit_pv():
                i, pAB, ot = pvq.pop(0)
                first, last = i == kept[0], i == kept[-1]
                nc.tensor.matmul(
                    pvA[0:65, ot:512], v_sb[:, 130 * i : 130 * i + 65],
                    pAB[:, ot:512], start=first, stop=last,
                )
                nc.tensor.matmul(
                    pvB[0:65, ot:512], v_sb[:, 130 * i + 65 : 130 * i + 130],
                    pAB[:, 512 + ot : 1024], start=first, stop=last,
                )

            # delay slot consumption: at(J-1) is produced by a normalize
            # chain that runs a few us past its last pv, so out-projection
            # slots emitted in the first chunks would stall the in-order PE.
            if tail:
                skip = min(len(kept) - 1, 6)
            else:
                skip = 2 if len(kept) > 6 else (1 if len(kept) > 4 else 0)
            nslot = (
                max(1, -(-len(slots) // (len(kept) - skip))) if slots else 0
            )
            for n, i in enumerate(kept):
                subs = [int(cls[i, 4 * J + u]) for u in range(4)]
                fk = min(u for u in range(4) if subs[u] != 1)
                o = 128 * fk
                psAB = psum.tile([128, 1024], f32, tag="sc", bufs=2)
                # masked-prefix trim: q-cols < o are fully masked for this
                # chunk; skip them in the score and pv matmuls (pv then
                # never reads the garbage probs there).  The first kept
                # chunk is never trimmed so its start=True zeroes the full
                # pv region.
                ot = o if n > 0 else 0
                isl = slice(128 * i, 128 * (i + 1))
                jsl = slice(512 * J + ot, 512 * (J + 1))
                if SCORES_MODE == "fp8":
                    nc.tensor.matmul(
                        psAB[:, ot:512], kk[:, 0:2, isl], qq[:, 0:2, jsl],
                        start=True, stop=True, perf_mode=DR,
                    )
                    nc.tensor.matmul(
                        psAB[:, 512 + ot : 1024], kk[:, 2:4, isl],
                        qq[:, 2:4, jsl],
                        start=True, stop=True, perf_mode=DR,
                    )
                else:
                    nc.tensor.matmul(
                        psAB[:, ot:512], kT2[0:64, isl], qT2[0:64, jsl],
                        start=True, stop=True, tile_position=(0, 0),
                    )
                    nc.tensor.matmul(
                        psAB[:, 512 + ot : 1024], kT2[64:128, isl],
                        qT2[64:128, jsl],
                        start=True, stop=True, tile_position=(64, 0),
                    )
                if n >= skip:
                    for _ in range(nslot):
                        if slots:
                            slots.pop(0)()
                if len(pvq) >= 2:
                    emit_pv()
                for u, cu in enumerate(subs):
                    if cu == 2:
                        gi = GEN_INDEX[(i, 4 * J + u)]
                        # one strided add covers head A and head B blocks
                        pview = psAB[:].rearrange("p (h w) -> p h w", h=2)[
                            :, :, 128 * u : 128 * (u + 1)
                        ]
                        mview = c_maskg[:, None, 128 * gi : 128 * (gi + 1)]
                        nc.vector.tensor_add(
                            pview, pview, mview.broadcast_to([128, 2, 128])
                        )
                # one exp covers both heads; masked subs get zero-filled after
                pAB = probs_pool.tile([128, 1024], bf16)
                if ot == 0:
                    nc.scalar.activation(
                        pAB[:, 0:1024], psAB[:, 0:1024], Exp,
                        bias=c_bias[:, 0:1], scale=sc_scale,
                    )
                else:  # trimmed chunk: the strip between the heads was
                    # never written; exp each head's valid region alone
                    nc.scalar.activation(
                        pAB[:, ot:512], psAB[:, ot:512], Exp,
                        bias=c_bias[:, 0:1], scale=sc_scale,
                    )
                    nc.scalar.activation(
                        pAB[:, 512 + ot : 1024], psAB[:, 512 + ot : 1024],
                        Exp, bias=c_bias[:, 0:1], scale=sc_scale,
                    )
                for u, cu in enumerate(subs):
                    if cu == 1 and u > fk:
                        for off in (0, 512):
                            usl = slice(off + 128 * u, off + 128 * (u + 1))
                            (nc.gpsimd if POOL_MODE == "on" else nc.vector).tensor_copy(
                                pAB[:, usl], c_one2[:, 2:130]
                            )
                pvq.append((i, pAB, ot))
            while pvq:
                emit_pv()
            while slots:
                slots.pop(0)()
            # normalize:  attn = pv[0:64] * (1/denom row 64); head B first
            # (it needs an extra SBUF->SBUF partition-shift DMA into
            # at[64:128] so the output projection can contract both heads
            # in one K=128 matmul).  The reciprocal runs directly on
            # partition 64; a K=1 fp32 matmul broadcasts it to 64 rows.
            at = attn_pool.tile([128, 512], bf16)
            for hb, pv in ((1, pvB), (0, pvA)):
                # copy PSUM->SBUF immediately so the pv bank frees for the
                # next J before the reciprocal chain runs.
                pvf = wk_pool.tile([65, 512], f32, tag="den", bufs=2)
                nc.vector.tensor_copy(pvf[64:65, :], pv[64:65, :])
                if NORM_MODE == "v3":
                    d0 = wk_pool.tile([1, 512], f32, tag="d0", bufs=4)
                    nc.sync.dma_start(d0[:], pvf[64:65, :])
                nc.vector.tensor_copy(pvf[0:64, :], pv[0:64, :])
                if NORM_MODE == "pe":
                    r0 = wk_pool.tile([65, 512], f32, tag="d0", bufs=4)
                    s0 = wk_pool.tile([65, 512], f32, tag="d0", bufs=4)
                    nc.vector.reciprocal_approx_accurate(
                        r0[64:65, :], pvf[64:65, :], s0[64:65, :]
                    )
                    bc = psum.tile([64, 512], f32, tag="proj", bufs=2)
                    nc.tensor.matmul(
                        bc[:], c_ones64[64:65, 0:64], r0[64:65, :],
                        start=True, stop=True, tile_position=(64, 0),
                    )
                    bcin = bc[:]
                elif NORM_MODE == "direct":
                    r0 = wk_pool.tile([65, 512], f32, tag="d0", bufs=4)
                    s0 = wk_pool.tile([65, 512], f32, tag="d0", bufs=4)
                    nc.vector.reciprocal_approx_accurate(
                        r0[64:65, :], pvf[64:65, :], s0[64:65, :]
                    )
                    bcs = wk_pool.tile([64, 512], f32, tag="bcs", bufs=2)
                    nc.gpsimd.partition_broadcast(bcs[:], r0[64:65, :])
                    bcin = bcs[:]
                else:
                    r0 = wk_pool.tile([1, 512], f32, tag="d0", bufs=4)
                    s0 = wk_pool.tile([1, 512], f32, tag="d0", bufs=4)
                    nc.vector.reciprocal_approx_accurate(r0[:], d0[:], s0[:])
                    bcs = wk_pool.tile([64, 512], f32, tag="bcs", bufs=2)
                    nc.gpsimd.partition_broadcast(bcs[:], r0[:])
                    bcin = bcs[:]
                if hb:
                    atB = attn_pool.tile([64, 512], bf16)
                    nc.vector.tensor_mul(atB[:], pvf[0:64, :], bcin)
                    nc.sync.dma_start(at[64:128, :], atB[:])
                else:
                    nc.vector.tensor_mul(at[0:64, :], pvf[0:64, :], bcin)
            # build outproj slot closures for this J (emitted during J+1)
            yo = yo_pool.tile([128, 4096], bf16)
            nleft = [8]

            def mk(scn, do2):
                def emit(use_alt=False):
                    ssl2 = slice(128 * scn, 128 * (scn + 1))
                    dsl = slice(512 * do2, 512 * (do2 + 1))
                    pso = psum.tile([128, 512], f32, tag="proj", bufs=2)
                    nc.tensor.matmul(
                        pso[:], at[:, ssl2], c_wo[:, dsl], start=True, stop=True
                    )
                    osl = slice(
                        1024 * scn + 512 * do2, 1024 * scn + 512 * do2 + 512
                    )
                    if use_alt:
                        nc.scalar.copy(yo[:, osl], pso[:])
                    else:
                        nc.vector.tensor_copy(yo[:, osl], pso[:])
                    nleft[0] -= 1
                    # slots emit scn-major, so the first half of yo is
                    # complete at nleft==4: drain it early
                    if nleft[0] == 4:
                        nc.sync.dma_start(
                            yP[:, 4096 * J : 4096 * J + 2048], yo[:, 0:2048]
                        )
                    elif nleft[0] == 0:
                        nc.sync.dma_start(
                            yP[:, 4096 * J + 2048 : 4096 * (J + 1)],
                            yo[:, 2048:4096],
                        )

                return emit

            return [mk(scn, do2) for scn in range(4) for do2 in range(2)]

        # attn(J) reads k/v blocks up to max(kept); interleaving phase1/attn
        # is only legal when that never exceeds the last-written block.
        # attn(J) is emitted after phase1(J+1) because even-numbered blocks
        # only flush to qf8/kf8 together with the following odd block.
        causal_ok = all(
            i <= 4 * J + 3
            for J in range(NJ)
            for i in range(nb)
            if any(cls[i, 4 * J + u] != 1 for u in range(4))
        )
        slots = []
        if causal_ok and NJ > 1 and SCORES_MODE == "fp8":
            # attn(0) is the smallest block: run it LAST so the pipeline
            # drains with the least unoverlappable softmax work.  fp8 mode
            # needs attn(J) after phase1(J+1) for the block-pair flush.
            phase1(0)
            phase1(1)
            late_consts()
            for J in range(1, NJ):
                if J + 1 < NJ:
                    phase1(J + 1)
                slots = attn(J, slots)
            slots = attn(0, slots)
        elif causal_ok and NJ > 2:
            # attn order a1, a3..a7, a2, a0: the two tail blocks (no
            # phase1 left to interleave) are mid-sized so each hides the
            # previous block's normalize chain + outproj slots.
            phase1(0)
            phase1(1)
            late_consts()
            slots = attn(1, slots)
            phase1(2)
            for sb in range(3, NJ):
                phase1(sb)
                slots = attn(sb, slots)
            slots = attn(2, slots, tail=True)
            slots = attn(0, slots, tail=True)
        elif causal_ok and NJ > 1:
            phase1(0)
            phase1(1)
            late_consts()
            slots = attn(1, slots)
            slots = attn(0, slots, tail=True)
        else:
            for sb in range(NJ):
                phase1(sb)
                if sb == min(1, NJ - 1):
                    late_consts()
            for J in range(NJ):
                slots = attn(J, slots)
        for n, emit in enumerate(slots):
            emit(use_alt=n % 2 == 1)

    nc.compile()
    return nc


GEN_INDEX = {}


def host_prep(x, freqs_cos, freqs_sin, mask, wq, wk, wv, wo):
    """Build per-core input maps + mask classification.  Returns
    (in_maps, cls, n_gen, neg_bias, sc_scale)."""
    global GEN_INDEX
    import ml_dtypes

    bf16 = ml_dtypes.bfloat16
    s = x.shape[1]
    nd, NJ_ = D // 128, s // 512
    x2 = np.ascontiguousarray(x.reshape(s, D))
    xT = np.ascontiguousarray(x2.T).astype(bf16)
    # packed layout: xP[p, sb*nd*512 + dc*512 + sl] = xT[128*dc+p, 512*sb+sl]
    xP = np.ascontiguousarray(
        xT.reshape(nd, 128, NJ_, 512).transpose(1, 2, 0, 3).reshape(128, nd * s)
    )

    # rope tables in T layout (same for q and k; scales folded into wq/wk)
    p = np.arange(128)
    j = (p % HD) // 2  # freq index per partition row
    cosT = np.ascontiguousarray(freqs_cos.T[j, :]).astype(bf16)  # [128, s]
    sinT = freqs_sin.T[j, :]
    sign = np.where(p % 2 == 0, -1.0, 1.0).astype(np.float32)
    sinTS = np.ascontiguousarray(sinT * sign[:, None]).astype(bf16)
    # cos|sin interleaved per 512-chunk: csP[p, sb*1024 + {0:512 cos, 512:1024 sin}]
    csP = np.ascontiguousarray(
        np.concatenate(
            [cosT.reshape(128, NJ_, 512), sinTS.reshape(128, NJ_, 512)], axis=2
        ).reshape(128, 2 * s)
    )

    # swap-adjacent permutation and identity
    P = np.zeros((128, 128), np.float32)
    P[np.arange(128) ^ 1, np.arange(128)] = 1.0
    I = np.eye(128, dtype=np.float32)

    # softmax shift + fp8 scale factors, from exact host projections
    qf = x2 @ wq.T
    kf = x2 @ wk.T
    qh = _rope_np(qf, freqs_cos, freqs_sin)
    kh = _rope_np(kf, freqs_cos, freqs_sin)
    m = 0.0
    for h in range(H):
        qs = qh[:, HD * h : HD * (h + 1)]
        ks = kh[:, HD * h : HD * (h + 1)]
        m = max(m, float(np.abs(qs @ ks.T).max()) / 8.0)
    neg_bias = max(0.0, m - 60.0)
    sQ = 224.0 / max(float(np.abs(qh).max()) * 0.125, 1e-30)
    sK = 224.0 / max(float(np.abs(kh).max()), 1e-30)
    sc_scale = 1.0 / (sQ * sK)

    # mask classification + general block packing (mask values pre-scaled
    # by sQ*sK so the activation scale recovers them exactly)
    maskT = np.ascontiguousarray(mask.T).astype(np.float32)
    cls = classify_mask(maskT, s)
    GEN_INDEX = {}
    gen_blocks = []
    nbk = s // 128
    for i in range(nbk):
        for jj in range(nbk):
            if cls[i, jj] == 2:
                GEN_INDEX[(i, jj)] = len(gen_blocks)
                gen_blocks.append(
                    maskT[128 * i : 128 * (i + 1), 128 * jj : 128 * (jj + 1)]
                    * (sQ * sK)
                )
    n_gen = len(gen_blocks)
    if n_gen:
        maskg = np.ascontiguousarray(np.concatenate(gen_blocks, axis=1))
        maskg = np.maximum(maskg, -3e38)
    else:
        maskg = np.zeros((128, 128), np.float32)

    def pack_w(w_slice):  # [EC, D] -> [128, D] chunked-transpose layout
        t = np.ascontiguousarray(w_slice.T)  # [D, EC=128]
        return np.ascontiguousarray(
            t.reshape(D // 128, 128, 128).transpose(1, 0, 2).reshape(128, D)
        ).astype(bf16)

    ones130 = np.zeros((128, 130), np.float32)
    ones130[:, 0:2] = 1.0
    ones64 = np.ones((128, 64), np.float32)
    biasC = np.full((128, 1), -neg_bias, np.float32)
    in_maps = []
    for c in range(NCORES):
        R = slice(EC * c, EC * (c + 1))
        woc = wo[:, R]  # [D, 128]
        woT_pack = np.ascontiguousarray(woc.T).astype(bf16)  # [128, D]
        in_maps.append(
            {
                "xP": xP,
                "csP": csP,
                "wqT": pack_w(wq[R] * (0.125 * sQ)),
                "wkT": pack_w(wk[R] * sK),
                "wvT": pack_w(wv[R]),
                "woT": woT_pack,
                "P128": P.astype(bf16),
                "I128": I.astype(bf16),
                "maskg": maskg,
                "ones2": ones130.astype(bf16),
                "ones64": ones64,
                "biasC": biasC,
            }
        )
    return in_maps, cls, n_gen, neg_bias, sc_scale


def _rope_np(t, cos, sin):
    s = t.shape[0]
    tr = t.reshape(s, H, HD // 2, 2)
    te, to = tr[..., 0], tr[..., 1]
    c = cos[:, None, :]
    sn = sin[:, None, :]
    oe = te * c - to * sn
    oo = te * sn + to * c
    return np.stack([oe, oo], axis=-1).reshape(s, H * HD)


def kernel(**inputs):
    from concourse.bass_utils import run_bass_kernel_spmd

    x = np.asarray(inputs["x"], np.float32)
    in_maps, cls, n_gen, neg_bias, sc_scale = host_prep(
        x,
        np.asarray(inputs["freqs_cos"], np.float32),
        np.asarray(inputs["freqs_sin"], np.float32),
        np.asarray(inputs["mask"], np.float32),
        np.asarray(inputs["wq"], np.float32),
        np.asarray(inputs["wk"], np.float32),
        np.asarray(inputs["wv"], np.float32),
        np.asarray(inputs["wo"], np.float32),
    )
    key = (x.shape[1], cls.tobytes(), n_gen, float(neg_bias), float(sc_scale))
    if key not in _PROGRAM_CACHE:
        _PROGRAM_CACHE[key] = build_program(
            x.shape[1], cls, n_gen, neg_bias, sc_scale
        )
    nc = _PROGRAM_CACHE[key]
    res = run_bass_kernel_spmd(nc, in_maps, core_ids=list(range(NCORES)))
    s = x.shape[1]
    yP = np.zeros((128, (s // 512) * 4096), np.float32)
    for c in range(NCORES):
        yP += np.asarray(res.results[c]["yP"], np.float32)
    # yP[p, J*4096 + scn*1024 + d] = y[512J + 128*scn + p, d]
    y = yP.reshape(128, s // 512, 4, D).transpose(1, 2, 0, 3).reshape(s, D)
    return np.ascontiguousarray(y).reshape(x.shape)


# revision 29
# speedup vs baseline: 1.0296x; 1.0296x over previous
"""Trainium2 Bass kernel for causal multi-head attention with RoPE.

Problem: B=1, S=4096, D=1024, H=16 heads of HD=64.
  q/k/v = x @ w{q,k,v}.T ; rope(q), rope(k); scores = q k^T/sqrt(HD) + mask;
  out = softmax(scores) @ v ; y = out @ wo.T

Sharding: tensor-parallel over heads. 8 cores x 2 heads each.  Each core
computes its 2 heads' q/k/v projections (column-split weights), full
attention for those heads over all 4096 positions, and a partial output
projection (row-split wo).  The host sums the 8 partial [S, D] outputs.

v5 structure (ACT-engine exp is the roofline; fp8 DoubleRow halves the
score matmuls; probs/v/pv stay bf16 for accuracy):
  - q/k (after rope) are stored fp8e4 with host-folded scales in a
    [32, 4S] layout (4 k-tiles: headA-lo, headA-hi, headB-lo, headB-hi);
    score matmuls run in MatmulPerfMode.DoubleRow at 0.5 cycles/row,
    contracting hd=64 as 2 k-tiles of 32.  exp() recovers the scale via
    the activation scale operand; the mask is pre-scaled on the host.
  - The rope output is written fp8 into a 2-block staging tile and
    flushed to the [32, 4S] layout by partition-remap DMAs each block
    pair; attn(J) is therefore emitted after phase1(J+1).
  - probs (bf16) / v (bf16 with ones column) / pv matmuls are the v3
    scheme: two heads share one [128, 1024] psum score tile, one exp
    covers both, pv accumulates [65, 512] per head with the denominator
    in row 64.
  - Engine rebalance: psum->sbuf casts, rope cos-mul, mask adds and
    prob zero-fills run on Pool (gpsimd) instead of DVE; the softmax
    reciprocal runs directly on partition 64 and is broadcast by a K=1
    fp32 PE matmul (no partition-move DMAs in the normalize chain);
    head B is normalized first so its extra SBUF shift DMA overlaps
    head A's chain.
  - Startup DMAs are spread across the gpsimd + sync + scalar hardware
    queues with the first projection block's chunks first.
  - y partials are written bf16, one consolidated DMA per 512-row
    block; outproj slots interleave into the next J's score chunks.
"""

import os
import sys

import numpy as np

sys.path.insert(0, "/opt/trn_rl_repo")

S = 4096
D = 1024
H = 16
HD = 64
NCORES = 8
HPC = H // NCORES  # 2 heads per core
EC = HPC * HD  # 128 head-dim columns per core
NEG_THRESH = -1e8  # blocks entirely <= this are "fully masked"

_PROGRAM_CACHE = {}
SCORES_MODE = os.environ.get("V5_SCORES", "bf16")  # fp8 | bf16
NORM_MODE = os.environ.get("V5_NORM", "v3")  # pe | v3 | direct
POOL_MODE = os.environ.get("V5_POOL", "off")  # on | off (gpsimd tensor ops)


def classify_mask(maskT, s):
    """Classify 128x128 blocks of mask.T: 0=zero, 1=neginf, 2=general."""
    nb = s // 128
    cls = np.zeros((nb, nb), np.int8)
    for i in range(nb):
        for j in range(nb):
            blk = maskT[128 * i : 128 * (i + 1), 128 * j : 128 * (j + 1)]
            if np.all(blk == 0.0):
                cls[i, j] = 0
            elif np.all(blk <= NEG_THRESH):
                cls[i, j] = 1
            else:
                cls[i, j] = 2
    return cls


def build_program(s, cls, n_gen, neg_bias, sc_scale):
    """Build the SPMD Bass/Tile program for one core (same for all cores)."""
    from contextlib import ExitStack

    import concourse.bass as bass
    import concourse.tile as tile
    from concourse import bacc, mybir

    f32 = mybir.dt.float32
    bf16 = mybir.dt.bfloat16
    fp8 = mybir.dt.float8e4
    Exp = mybir.ActivationFunctionType.Exp
    DR = mybir.MatmulPerfMode.DoubleRow

    nb = s // 128  # sk chunks
    NJ = s // 512  # sq blocks
    nd = D // 128  # contraction chunks for projections

    nc = bacc.Bacc(
        "TRN2", target_bir_lowering=False, debug=False, num_devices=NCORES
    )

    # xP / csP / yP are host-shuffled so every per-sb DMA is a plain
    # contiguous [128, N] block.
    xP = nc.dram_tensor("xP", [128, (D // 128) * s], bf16, kind="ExternalInput").ap()
    csP = nc.dram_tensor("csP", [128, 2 * s], bf16, kind="ExternalInput").ap()
    wqT = nc.dram_tensor("wqT", [128, D], bf16, kind="ExternalInput").ap()
    wkT = nc.dram_tensor("wkT", [128, D], bf16, kind="ExternalInput").ap()
    wvT = nc.dram_tensor("wvT", [128, D], bf16, kind="ExternalInput").ap()
    woT = nc.dram_tensor("woT", [128, D], bf16, kind="ExternalInput").ap()
    P128 = nc.dram_tensor("P128", [128, 128], bf16, kind="ExternalInput").ap()
    I128 = nc.dram_tensor("I128", [128, 128], bf16, kind="ExternalInput").ap()
    maskg = nc.dram_tensor(
        "maskg", [128, 128 * max(n_gen, 1)], f32, kind="ExternalInput"
    ).ap()
    ones2 = nc.dram_tensor("ones2", [128, 130], bf16, kind="ExternalInput").ap()
    ones64 = nc.dram_tensor("ones64", [128, 64], f32, kind="ExternalInput").ap()
    biasC = nc.dram_tensor("biasC", [128, 1], f32, kind="ExternalInput").ap()
    yP = nc.dram_tensor("yP", [128, (s // 512) * 4096], bf16, kind="ExternalOutput").ap()

    with tile.TileContext(nc) as tc, ExitStack() as ctx:
        consts = ctx.enter_context(tc.tile_pool(name="consts", bufs=1))
        persist = ctx.enter_context(tc.tile_pool(name="persist", bufs=1))
        xt_pool = ctx.enter_context(tc.tile_pool(name="xt", bufs=3))
        wk_pool = ctx.enter_context(tc.tile_pool(name="work", bufs=6))
        probs_pool = ctx.enter_context(tc.tile_pool(name="probs", bufs=6))
        attn_pool = ctx.enter_context(tc.tile_pool(name="attn", bufs=3))
        yo_pool = ctx.enter_context(tc.tile_pool(name="yo", bufs=2))
        psum = ctx.enter_context(tc.tile_pool(name="psum", bufs=2, space="PSUM"))

        # ---- constants, spread across the three DMA queues.  gpsimd gets
        # everything phase1(0)'s q-pass touches (wq + x dc 0..3); sync gets
        # x dc 4..7; scalar gets the rest of phase1(0)'s needs.
        c_wq = consts.tile([128, D], bf16)
        c_cs = consts.tile([128, 2 * s], bf16)
        xts0 = xt_pool.tile([128, nd * 512], bf16)
        for dc in range(nd):
            if dc < 4:
                nc.gpsimd.dma_start(
                    c_wq[:, 128 * dc : 128 * (dc + 1)],
                    wqT[:, 128 * dc : 128 * (dc + 1)],
                )
                nc.gpsimd.dma_start(
                    xts0[:, 512 * dc : 512 * (dc + 1)],
                    xP[:, 512 * dc : 512 * (dc + 1)],
                )
            else:
                nc.sync.dma_start(
                    xts0[:, 512 * dc : 512 * (dc + 1)],
                    xP[:, 512 * dc : 512 * (dc + 1)],
                )
        nc.gpsimd.dma_start(c_wq[:, 512:1024], wqT[:, 512:1024])
        nc.sync.dma_start(c_cs[:, 0:1024], csP[:, 0:1024])
        c_P = consts.tile([128, 128], bf16)
        nc.sync.dma_start(c_P[:], P128[:])
        c_wk = consts.tile([128, D], bf16)
        nc.sync.dma_start(c_wk[:], wkT[:])
        c_wv = consts.tile([128, D], bf16)
        nc.sync.dma_start(c_wv[:], wvT[:])
        c_I = consts.tile([128, 128], bf16)
        nc.sync.dma_start(c_I[:], I128[:])
        c_one2 = consts.tile([128, 130], bf16)  # cols 0:2 ones, 2:130 zeros
        nc.gpsimd.dma_start(c_one2[:], ones2[:])
        c_ones64 = consts.tile([128, 64], f32)
        nc.gpsimd.dma_start(c_ones64[:], ones64[:])
        c_bias = consts.tile([128, 1], f32)
        nc.gpsimd.dma_start(c_bias[:], biasC[:])
        # maskg (2MB) and wo are not needed until ~15us in; they go on the
        # scalar queue after the early blocks so they don't block x/cs.
        c_maskg = consts.tile([128, 128 * max(n_gen, 1)], f32)
        c_wo = consts.tile([128, D], bf16)

        def late_consts():
            nc.scalar.dma_start(c_maskg[:], maskg[:])
            nc.scalar.dma_start(c_wo[:], woT[:])

        # ---- persistent activations ----
        # qf8/kf8: [32, 4S] fp8.  col t*S+c holds, for partition p:
        #   t=0: head A rope'd dim p      at position c
        #   t=1: head A rope'd dim 32+p
        #   t=2: head B rope'd dim p
        #   t=3: head B rope'd dim 32+p
        v_sb = persist.tile([128, nb * 130], bf16)  # per sk-chunk: [A 64|1|B 64|1]
        if SCORES_MODE == "fp8":
            qf8 = persist.tile([32, 4 * s], fp8)
            kf8 = persist.tile([32, 4 * s], fp8)
            qq = qf8[:].rearrange("p (t c) -> p t c", c=s)
            kk = kf8[:].rearrange("p (t c) -> p t c", c=s)
        else:
            qT2 = persist.tile([128, s], bf16)
            kT2 = persist.tile([128, s], bf16)
        stg_cur = {}  # live fp8 staging tile per tensor ("q"/"k")
        xts_tiles = {}

        def load_xts(sb):
            """Prefetch x block sb + its cos/sin chunk (emitted ~2 blocks
            ahead so the 1MB transfer hides under compute)."""
            if sb <= 0 or sb >= NJ or sb in xts_tiles:
                return
            ceng = nc.scalar if sb < 2 else nc.sync
            ceng.dma_start(c_cs[:, 1024 * sb : 1024 * (sb + 1)],
                           csP[:, 1024 * sb : 1024 * (sb + 1)])
            t = xt_pool.tile([128, nd * 512], bf16, name="xtsp")
            nc.sync.dma_start(t[:], xP[:, nd * 512 * sb : nd * 512 * (sb + 1)])
            xts_tiles[sb] = t

        # ones columns of v_aug (cols 64 and 129 of each 130-wide chunk)
        ones_view = v_sb[:].rearrange("p (c w) -> p c w", w=130)[:, :, 64:130:65]
        (nc.gpsimd if POOL_MODE == "on" else nc.vector).tensor_copy(
            ones_view, c_one2[:, None, 0:2].broadcast_to([128, nb, 2])
        )

        def phase1(sb):
            """Projections + rope + v-transpose for rows [512*sb, 512*sb+512).
            q/k land (via rope) fp8 in a 2-block staging tile, flushed to
            qf8/kf8 with partition-remap DMAs each block pair."""
            csl = slice(1024 * sb, 1024 * sb + 512)  # cos chunk in c_cs
            snl = slice(1024 * sb + 512, 1024 * (sb + 1))  # sin chunk
            xts = xts_tiles.pop(sb) if sb else xts0
            so = 512 * (sb % 2)  # column offset in the 2-block staging tiles
            for cw, which in ((c_wq, "q"), (c_wk, "k")):
                ps = psum.tile([128, 512], f32, tag="proj", bufs=2)
                for dc in range(nd):
                    nc.tensor.matmul(
                        ps[:], cw[:, 128 * dc : 128 * (dc + 1)],
                        xts[:, 512 * dc : 512 * (dc + 1)],
                        start=dc == 0, stop=dc == nd - 1,
                    )
                raw = wk_pool.tile([128, 512], bf16, tag="rope")
                nc.vector.tensor_copy(raw[:], ps[:])
                psw = psum.tile([128, 512], f32, tag="proj", bufs=2)
                nc.tensor.matmul(psw[:], c_P[:], raw[:], start=True, stop=True)
                t1 = wk_pool.tile([128, 512], bf16, tag="rope")
                peng = nc.gpsimd if POOL_MODE == "on" else nc.vector
                peng.tensor_mul(t1[:], raw[:], c_cs[:, csl])
                t2 = wk_pool.tile([128, 512], bf16, tag="rope")
                nc.vector.tensor_mul(t2[:], psw[:], c_cs[:, snl])
                if SCORES_MODE != "fp8":
                    dstT = qT2 if which == "q" else kT2
                    nc.vector.tensor_add(
                        dstT[:, 512 * sb : 512 * (sb + 1)], t1[:], t2[:]
                    )
                    continue
                if sb % 2 == 0:
                    stg_cur[which] = wk_pool.tile(
                        [128, 1024], fp8, tag="stg" + which, bufs=2,
                        name="stg" + which,
                    )
                stg = stg_cur[which]
                nc.vector.tensor_add(stg[:, so : so + 512], t1[:], t2[:])
                # flush staged blocks to the [32, 4S] DoubleRow layout
                if sb % 2 == 1 or sb == NJ - 1:
                    w = so + 512
                    c0 = 512 * (sb - sb % 2)
                    dst = qq if which == "q" else kk
                    deng = nc.sync
                    for t in range(4):
                        deng.dma_start(
                            dst[:, t, c0 : c0 + w],
                            stg[32 * t : 32 * (t + 1), 0:w],
                        )
            # v pass: psum -> sbuf, then per-128 transpose into natural layout
            psv = psum.tile([128, 512], f32, tag="proj", bufs=2)
            for dc in range(nd):
                nc.tensor.matmul(
                    psv[:], c_wv[:, 128 * dc : 128 * (dc + 1)],
                    xts[:, 512 * dc : 512 * (dc + 1)],
                    start=dc == 0, stop=dc == nd - 1,
                )
            vtt = wk_pool.tile([128, 512], bf16, tag="vtt", bufs=2)
            nc.vector.tensor_copy(vtt[:], psv[:])
            for k4 in range(4):
                sc = 4 * sb + k4
                pst = psum.tile([128, 1024], bf16, tag="proj", bufs=2)
                nc.tensor.transpose(
                    pst[:, 0:128], vtt[:, 128 * k4 : 128 * (k4 + 1)], c_I[:]
                )
                vdst = v_sb[:].rearrange("p (c w) -> p c w", w=65)[
                    :, 2 * sc : 2 * sc + 2, 0:64
                ]
                psrc = pst[:, 0:128].rearrange("p (c w) -> p c w", w=64)
                nc.vector.tensor_copy(vdst, psrc)

        def attn(J, slots, tail=False):
            """Attention for q rows [512J, 512J+512); interleaves `slots`
            (previous J's output-projection emitters) between score chunks.
            Returns this J's outproj slots.  tail=True delays slot
            consumption further (the previous normalize has no phase1
            to hide behind)."""
            kept = [
                i
                for i in range(nb)
                if any(cls[i, 4 * J + u] != 1 for u in range(4))
            ]
            if not kept:
                while slots:
                    slots.pop(0)()
                return []
            pvA = psum.tile([128, 512], f32, tag="pv", bufs=2)
            pvB = psum.tile([128, 512], f32, tag="pv", bufs=2)
            pvq = []  # chunks whose pv matmuls are not yet emitted

            def emit_pv():
                i, pAB, ot = pvq.pop(0)
                first, last = i == kept[0], i == kept[-1]
                nc.tensor.matmul(
                    pvA[0:65, ot:512], v_sb[:, 130 * i : 130 * i + 65],
                    pAB[:, ot:512], start=first, stop=last,
                )
                nc.tensor.matmul(
                    pvB[0:65, ot:512], v_sb[:, 130 * i + 65 : 130 * i + 130],
                    pAB[:, 512 + ot : 1024], start=first, stop=last,
                )

            # delay slot consumption: at(J-1) is produced by a normalize
            # chain that runs a few us past its last pv, so out-projection
            # slots emitted in the first chunks would stall the in-order PE.
            if tail:
                skip = min(len(kept) - 1, 6)
            else:
                skip = 2 if len(kept) > 6 else (1 if len(kept) > 4 else 0)
            nslot = (
                max(1, -(-len(slots) // (len(kept) - skip))) if slots else 0
            )
            for n, i in enumerate(kept):
                subs = [int(cls[i, 4 * J + u]) for u in range(4)]
                fk = min(u for u in range(4) if subs[u] != 1)
                o = 128 * fk
                psAB = psum.tile([128, 1024], f32, tag="sc", bufs=2)
                # masked-prefix trim: q-cols < o are fully masked for this
                # chunk; skip them in the score and pv matmuls (pv then
                # never reads the garbage probs there).  The first kept
                # chunk is never trimmed so its start=True zeroes the full
                # pv region.
                ot = o if n > 0 else 0
                isl = slice(128 * i, 128 * (i + 1))
                jsl = slice(512 * J + ot, 512 * (J + 1))
                if SCORES_MODE == "fp8":
                    nc.tensor.matmul(
                        psAB[:, ot:512], kk[:, 0:2, isl], qq[:, 0:2, jsl],
                        start=True, stop=True, perf_mode=DR,
                    )
                    nc.tensor.matmul(
                        psAB[:, 512 + ot : 1024], kk[:, 2:4, isl],
                        qq[:, 2:4, jsl],
                        start=True, stop=True, perf_mode=DR,
                    )
                else:
                    nc.tensor.matmul(
                        psAB[:, ot:512], kT2[0:64, isl], qT2[0:64, jsl],
                        start=True, stop=True, tile_position=(0, 0),
                    )
                    nc.tensor.matmul(
                        psAB[:, 512 + ot : 1024], kT2[64:128, isl],
                        qT2[64:128, jsl],
                        start=True, stop=True, tile_position=(64, 0),
                    )
                if n >= skip:
                    for _ in range(nslot):
                        if slots:
                            slots.pop(0)()
                if len(pvq) >= 2:
                    emit_pv()
                for u, cu in enumerate(subs):
                    if cu == 2:
                        gi = GEN_INDEX[(i, 4 * J + u)]
                        # one strided add covers head A and head B blocks
                        pview = psAB[:].rearrange("p (h w) -> p h w", h=2)[
                            :, :, 128 * u : 128 * (u + 1)
                        ]
                        mview = c_maskg[:, None, 128 * gi : 128 * (gi + 1)]
                        nc.vector.tensor_add(
                            pview, pview, mview.broadcast_to([128, 2, 128])
                        )
                # one exp covers both heads; masked subs get zero-filled after
                pAB = probs_pool.tile([128, 1024], bf16)
                if ot == 0:
                    nc.scalar.activation(
                        pAB[:, 0:1024], psAB[:, 0:1024], Exp,
                        bias=c_bias[:, 0:1], scale=sc_scale,
                    )
                else:  # trimmed chunk: the strip between the heads was
                    # never written; exp each head's valid region alone
                    nc.scalar.activation(
                        pAB[:, ot:512], psAB[:, ot:512], Exp,
                        bias=c_bias[:, 0:1], scale=sc_scale,
                    )
                    nc.scalar.activation(
                        pAB[:, 512 + ot : 1024], psAB[:, 512 + ot : 1024],
                        Exp, bias=c_bias[:, 0:1], scale=sc_scale,
                    )
                for u, cu in enumerate(subs):
                    if cu == 1 and u > fk:
                        for off in (0, 512):
                            usl = slice(off + 128 * u, off + 128 * (u + 1))
                            (nc.gpsimd if POOL_MODE == "on" else nc.vector).tensor_copy(
                                pAB[:, usl], c_one2[:, 2:130]
                            )
                pvq.append((i, pAB, ot))
            while pvq:
                emit_pv()
            while slots:
                slots.pop(0)()
            # normalize:  attn = pv[0:64] * (1/denom row 64); head B first
            # (it needs an extra SBUF->SBUF partition-shift DMA into
            # at[64:128] so the output projection can contract both heads
            # in one K=128 matmul).  The reciprocal runs directly on
            # partition 64; a K=1 fp32 matmul broadcasts it to 64 rows.
            at = attn_pool.tile([128, 512], bf16)
            for hb, pv in ((1, pvB), (0, pvA)):
                # copy PSUM->SBUF immediately so the pv bank frees for the
                # next J before the reciprocal chain runs.
                pvf = wk_pool.tile([65, 512], f32, tag="den", bufs=2)
                nc.vector.tensor_copy(pvf[64:65, :], pv[64:65, :])
                if NORM_MODE == "v3":
                    d0 = wk_pool.tile([1, 512], f32, tag="d0", bufs=4)
                    nc.sync.dma_start(d0[:], pvf[64:65, :])
                nc.vector.tensor_copy(pvf[0:64, :], pv[0:64, :])
                if NORM_MODE == "pe":
                    r0 = wk_pool.tile([65, 512], f32, tag="d0", bufs=4)
                    s0 = wk_pool.tile([65, 512], f32, tag="d0", bufs=4)
                    nc.vector.reciprocal_approx_accurate(
                        r0[64:65, :], pvf[64:65, :], s0[64:65, :]
                    )
                    bc = psum.tile([64, 512], f32, tag="proj", bufs=2)
                    nc.tensor.matmul(
                        bc[:], c_ones64[64:65, 0:64], r0[64:65, :],
                        start=True, stop=True, tile_position=(64, 0),
                    )
                    bcin = bc[:]
                elif NORM_MODE == "direct":
                    r0 = wk_pool.tile([65, 512], f32, tag="d0", bufs=4)
                    s0 = wk_pool.tile([65, 512], f32, tag="d0", bufs=4)
                    nc.vector.reciprocal_approx_accurate(
                        r0[64:65, :], pvf[64:65, :], s0[64:65, :]
                    )
                    bcs = wk_pool.tile([64, 512], f32, tag="bcs", bufs=2)
                    nc.gpsimd.partition_broadcast(bcs[:], r0[64:65, :])
                    bcin = bcs[:]
                else:
                    r0 = wk_pool.tile([1, 512], f32, tag="d0", bufs=4)
                    s0 = wk_pool.tile([1, 512], f32, tag="d0", bufs=4)
                    nc.vector.reciprocal_approx_accurate(r0[:], d0[:], s0[:])
                    bcs = wk_pool.tile([64, 512], f32, tag="bcs", bufs=2)
                    nc.gpsimd.partition_broadcast(bcs[:], r0[:])
                    bcin = bcs[:]
                if hb:
                    atB = attn_pool.tile([64, 512], bf16)
                    nc.vector.tensor_mul(atB[:], pvf[0:64, :], bcin)
                    nc.sync.dma_start(at[64:128, :], atB[:])
                else:
                    nc.vector.tensor_mul(at[0:64, :], pvf[0:64, :], bcin)
            # build outproj slot closures for this J (emitted during J+1)
            yo = yo_pool.tile([128, 4096], bf16)
            nleft = [8]

            def mk(scn, do2):
                def emit(use_alt=False):
                    ssl2 = slice(128 * scn, 128 * (scn + 1))
                    dsl = slice(512 * do2, 512 * (do2 + 1))
                    pso = psum.tile([128, 512], f32, tag="proj", bufs=2)
                    nc.tensor.matmul(
                        pso[:], at[:, ssl2], c_wo[:, dsl], start=True, stop=True
                    )
                    osl = slice(
                        1024 * scn + 512 * do2, 1024 * scn + 512 * do2 + 512
                    )
                    if use_alt:
                        nc.scalar.copy(yo[:, osl], pso[:])
                    else:
                        nc.vector.tensor_copy(yo[:, osl], pso[:])
                    nleft[0] -= 1
                    # slots emit scn-major, so the first half of yo is
                    # complete at nleft==4: drain it early
                    if nleft[0] == 4:
                        nc.sync.dma_start(
                            yP[:, 4096 * J : 4096 * J + 2048], yo[:, 0:2048]
                        )
                    elif nleft[0] == 0:
                        nc.sync.dma_start(
                            yP[:, 4096 * J + 2048 : 4096 * (J + 1)],
                            yo[:, 2048:4096],
                        )

                return emit

            return [mk(scn, do2) for scn in range(4) for do2 in range(2)]

        # attn(J) reads k/v blocks up to max(kept); interleaving phase1/attn
        # is only legal when that never exceeds the last-written block.
        # attn(J) is emitted after phase1(J+1) because even-numbered blocks
        # only flush to qf8/kf8 together with the following odd block.
        causal_ok = all(
            i <= 4 * J + 3
            for J in range(NJ)
            for i in range(nb)
            if any(cls[i, 4 * J + u] != 1 for u in range(4))
        )
        slots = []
        if causal_ok and NJ > 1 and SCORES_MODE == "fp8":
            # attn(0) is the smallest block: run it LAST so the pipeline
            # drains with the least unoverlappable softmax work.  fp8 mode
            # needs attn(J) after phase1(J+1) for the block-pair flush.
            phase1(0)
            phase1(1)
            late_consts()
            for J in range(1, NJ):
                if J + 1 < NJ:
                    phase1(J + 1)
                slots = attn(J, slots)
            slots = attn(0, slots)
        elif causal_ok and NJ > 1:
            # v3 ordering: attn(sb) right after phase1(sb); attn(0) last.
            load_xts(1)
            phase1(0)
            load_xts(2)
            for sb in range(1, NJ):
                phase1(sb)
                if sb == 1:
                    late_consts()
                load_xts(sb + 2)
                slots = attn(sb, slots)
            slots = attn(0, slots)
        else:
            for sb in range(NJ):
                load_xts(sb)
                phase1(sb)
                if sb == min(1, NJ - 1):
                    late_consts()
                load_xts(sb + 1)
            for J in range(NJ):
                slots = attn(J, slots)
        for n, emit in enumerate(slots):
            emit(use_alt=n % 2 == 1)

    nc.compile()
    return nc


GEN_INDEX = {}


def host_prep(x, freqs_cos, freqs_sin, mask, wq, wk, wv, wo):
    """Build per-core input maps + mask classification.  Returns
    (in_maps, cls, n_gen, neg_bias, sc_scale)."""
    global GEN_INDEX
    import ml_dtypes

    bf16 = ml_dtypes.bfloat16
    s = x.shape[1]
    nd, NJ_ = D // 128, s // 512
    x2 = np.ascontiguousarray(x.reshape(s, D))
    xT = np.ascontiguousarray(x2.T).astype(bf16)
    # packed layout: xP[p, sb*nd*512 + dc*512 + sl] = xT[128*dc+p, 512*sb+sl]
    xP = np.ascontiguousarray(
        xT.reshape(nd, 128, NJ_, 512).transpose(1, 2, 0, 3).reshape(128, nd * s)
    )

    # rope tables in T layout (same for q and k; scales folded into wq/wk)
    p = np.arange(128)
    j = (p % HD) // 2  # freq index per partition row
    cosT = np.ascontiguousarray(freqs_cos.T[j, :]).astype(bf16)  # [128, s]
    sinT = freqs_sin.T[j, :]
    sign = np.where(p % 2 == 0, -1.0, 1.0).astype(np.float32)
    sinTS = np.ascontiguousarray(sinT * sign[:, None]).astype(bf16)
    # cos|sin interleaved per 512-chunk: csP[p, sb*1024 + {0:512 cos, 512:1024 sin}]
    csP = np.ascontiguousarray(
        np.concatenate(
            [cosT.reshape(128, NJ_, 512), sinTS.reshape(128, NJ_, 512)], axis=2
        ).reshape(128, 2 * s)
    )

    # swap-adjacent permutation and identity
    P = np.zeros((128, 128), np.float32)
    P[np.arange(128) ^ 1, np.arange(128)] = 1.0
    I = np.eye(128, dtype=np.float32)

    # softmax shift + fp8 scale factors, from exact host projections
    qf = x2 @ wq.T
    kf = x2 @ wk.T
    qh = _rope_np(qf, freqs_cos, freqs_sin)
    kh = _rope_np(kf, freqs_cos, freqs_sin)
    m = 0.0
    for h in range(H):
        qs = qh[:, HD * h : HD * (h + 1)]
        ks = kh[:, HD * h : HD * (h + 1)]
        m = max(m, float(np.abs(qs @ ks.T).max()) / 8.0)
    neg_bias = max(0.0, m - 60.0)
    sQ = 224.0 / max(float(np.abs(qh).max()) * 0.125, 1e-30)
    sK = 224.0 / max(float(np.abs(kh).max()), 1e-30)
    sc_scale = 1.0 / (sQ * sK)

    # mask classification + general block packing (mask values pre-scaled
    # by sQ*sK so the activation scale recovers them exactly)
    maskT = np.ascontiguousarray(mask.T).astype(np.float32)
    cls = classify_mask(maskT, s)
    GEN_INDEX = {}
    gen_blocks = []
    nbk = s // 128
    for i in range(nbk):
        for jj in range(nbk):
            if cls[i, jj] == 2:
                GEN_INDEX[(i, jj)] = len(gen_blocks)
                gen_blocks.append(
                    maskT[128 * i : 128 * (i + 1), 128 * jj : 128 * (jj + 1)]
                    * (sQ * sK)
                )
    n_gen = len(gen_blocks)
    if n_gen:
        maskg = np.ascontiguousarray(np.concatenate(gen_blocks, axis=1))
        maskg = np.maximum(maskg, -3e38)
    else:
        maskg = np.zeros((128, 128), np.float32)

    def pack_w(w_slice):  # [EC, D] -> [128, D] chunked-transpose layout
        t = np.ascontiguousarray(w_slice.T)  # [D, EC=128]
        return np.ascontiguousarray(
            t.reshape(D // 128, 128, 128).transpose(1, 0, 2).reshape(128, D)
        ).astype(bf16)

    ones130 = np.zeros((128, 130), np.float32)
    ones130[:, 0:2] = 1.0
    ones64 = np.ones((128, 64), np.float32)
    biasC = np.full((128, 1), -neg_bias, np.float32)
    in_maps = []
    for c in range(NCORES):
        R = slice(EC * c, EC * (c + 1))
        woc = wo[:, R]  # [D, 128]
        woT_pack = np.ascontiguousarray(woc.T).astype(bf16)  # [128, D]
        in_maps.append(
            {
                "xP": xP,
                "csP": csP,
                "wqT": pack_w(wq[R] * (0.125 * sQ)),
                "wkT": pack_w(wk[R] * sK),
                "wvT": pack_w(wv[R]),
                "woT": woT_pack,
                "P128": P.astype(bf16),
                "I128": I.astype(bf16),
                "maskg": maskg,
                "ones2": ones130.astype(bf16),
                "ones64": ones64,
                "biasC": biasC,
            }
        )
    return in_maps, cls, n_gen, neg_bias, sc_scale


def _rope_np(t, cos, sin):
    s = t.shape[0]
    tr = t.reshape(s, H, HD // 2, 2)
    te, to = tr[..., 0], tr[..., 1]
    c = cos[:, None, :]
    sn = sin[:, None, :]
    oe = te * c - to * sn
    oo = te * sn + to * c
    return np.stack([oe, oo], axis=-1).reshape(s, H * HD)


def kernel(**inputs):
    from concourse.bass_utils import run_bass_kernel_spmd

    x = np.asarray(inputs["x"], np.float32)
    in_maps, cls, n_gen, neg_bias, sc_scale = host_prep(
        x,
        np.asarray(inputs["freqs_cos"], np.float32),
        np.asarray(inputs["freqs_sin"], np.float32),
        np.asarray(inputs["mask"], np.float32),
        np.asarray(inputs["wq"], np.float32),
        np.asarray(inputs["wk"], np.float32),
        np.asarray(inputs["wv"], np.float32),
        np.asarray(inputs["wo"], np.float32),
    )
    key = (x.shape[1], cls.tobytes(), n_gen, float(neg_bias), float(sc_scale))
    if key not in _PROGRAM_CACHE:
        _PROGRAM_CACHE[key] = build_program(
            x.shape[1], cls, n_gen, neg_bias, sc_scale
        )
    nc = _PROGRAM_CACHE[key]
    res = run_bass_kernel_spmd(nc, in_maps, core_ids=list(range(NCORES)))
    s = x.shape[1]
    yP = np.zeros((128, (s // 512) * 4096), np.float32)
    for c in range(NCORES):
        yP += np.asarray(res.results[c]["yP"], np.float32)
    # yP[p, J*4096 + scn*1024 + d] = y[512J + 128*scn + p, d]
    y = yP.reshape(128, s // 512, 4, D).transpose(1, 2, 0, 3).reshape(s, D)
    return np.ascontiguousarray(y).reshape(x.shape)


# revision 34
# speedup vs baseline: 1.0632x; 1.0326x over previous
"""Trainium2 Bass kernel for causal multi-head attention with RoPE.

Problem: B=1, S=4096, D=1024, H=16 heads of HD=64.
  q/k/v = x @ w{q,k,v}.T ; rope(q), rope(k); scores = q k^T/sqrt(HD) + mask;
  out = softmax(scores) @ v ; y = out @ wo.T

Sharding: tensor-parallel over heads. 8 cores x 2 heads each.  Each core
computes its 2 heads' q/k/v projections (column-split weights), full
attention for those heads over all 4096 positions, and a partial output
projection (row-split wo).  The host sums the 8 partial [S, D] outputs.

v5 structure (ACT-engine exp is the roofline; fp8 DoubleRow halves the
score matmuls; probs/v/pv stay bf16 for accuracy):
  - q/k (after rope) are stored fp8e4 with host-folded scales in a
    [32, 4S] layout (4 k-tiles: headA-lo, headA-hi, headB-lo, headB-hi);
    score matmuls run in MatmulPerfMode.DoubleRow at 0.5 cycles/row,
    contracting hd=64 as 2 k-tiles of 32.  exp() recovers the scale via
    the activation scale operand; the mask is pre-scaled on the host.
  - The rope output is written fp8 into a 2-block staging tile and
    flushed to the [32, 4S] layout by partition-remap DMAs each block
    pair; attn(J) is therefore emitted after phase1(J+1).
  - probs (bf16) / v (bf16 with ones column) / pv matmuls are the v3
    scheme: two heads share one [128, 1024] psum score tile, one exp
    covers both, pv accumulates [65, 512] per head with the denominator
    in row 64.
  - Engine rebalance: psum->sbuf casts, rope cos-mul, mask adds and
    prob zero-fills run on Pool (gpsimd) instead of DVE; the softmax
    reciprocal runs directly on partition 64 and is broadcast by a K=1
    fp32 PE matmul (no partition-move DMAs in the normalize chain);
    head B is normalized first so its extra SBUF shift DMA overlaps
    head A's chain.
  - Startup DMAs are spread across the gpsimd + sync + scalar hardware
    queues with the first projection block's chunks first.
  - y partials are written bf16, one consolidated DMA per 512-row
    block; outproj slots interleave into the next J's score chunks.
"""

import os
import sys

import numpy as np

sys.path.insert(0, "/opt/trn_rl_repo")

S = 4096
D = 1024
H = 16
HD = 64
NCORES = 8
HPC = H // NCORES  # 2 heads per core
EC = HPC * HD  # 128 head-dim columns per core
NEG_THRESH = -1e8  # blocks entirely <= this are "fully masked"

_PROGRAM_CACHE = {}
SCORES_MODE = os.environ.get("V5_SCORES", "bf16")  # fp8 | bf16
NORM_MODE = os.environ.get("V5_NORM", "v3")  # pe | v3 | direct
POOL_MODE = os.environ.get("V5_POOL", "off")  # on | off (gpsimd tensor ops)


def classify_mask(maskT, s):
    """Classify 128x128 blocks of mask.T: 0=zero, 1=neginf, 2=general."""
    nb = s // 128
    cls = np.zeros((nb, nb), np.int8)
    for i in range(nb):
        for j in range(nb):
            blk = maskT[128 * i : 128 * (i + 1), 128 * j : 128 * (j + 1)]
            if np.all(blk == 0.0):
                cls[i, j] = 0
            elif np.all(blk <= NEG_THRESH):
                cls[i, j] = 1
            else:
                cls[i, j] = 2
    return cls


def build_program(s, cls, n_gen, neg_bias, sc_scale):
    """Build the SPMD Bass/Tile program for one core (same for all cores)."""
    from contextlib import ExitStack

    import concourse.bass as bass
    import concourse.tile as tile
    from concourse import bacc, mybir

    f32 = mybir.dt.float32
    bf16 = mybir.dt.bfloat16
    fp8 = mybir.dt.float8e4
    Exp = mybir.ActivationFunctionType.Exp
    DR = mybir.MatmulPerfMode.DoubleRow

    nb = s // 128  # sk chunks
    NJ = s // 512  # sq blocks
    nd = D // 128  # contraction chunks for projections

    nc = bacc.Bacc(
        "TRN2", target_bir_lowering=False, debug=False, num_devices=NCORES
    )

    # xP / csP / yP are host-shuffled so every per-sb DMA is a plain
    # contiguous [128, N] block.
    xP = nc.dram_tensor("xP", [128, (D // 128) * s], bf16, kind="ExternalInput").ap()
    csP = nc.dram_tensor("csP", [128, 2 * s], bf16, kind="ExternalInput").ap()
    wqT = nc.dram_tensor("wqT", [128, D], bf16, kind="ExternalInput").ap()
    wkT = nc.dram_tensor("wkT", [128, D], bf16, kind="ExternalInput").ap()
    wvT = nc.dram_tensor("wvT", [128, D], bf16, kind="ExternalInput").ap()
    woT = nc.dram_tensor("woT", [128, D], bf16, kind="ExternalInput").ap()
    P128 = nc.dram_tensor("P128", [128, 128], bf16, kind="ExternalInput").ap()
    I128 = nc.dram_tensor("I128", [128, 128], bf16, kind="ExternalInput").ap()
    maskg = nc.dram_tensor(
        "maskg", [128, 128 * max(n_gen, 1)], f32, kind="ExternalInput"
    ).ap()
    ones2 = nc.dram_tensor("ones2", [128, 130], bf16, kind="ExternalInput").ap()
    ones64 = nc.dram_tensor("ones64", [128, 64], f32, kind="ExternalInput").ap()
    biasC = nc.dram_tensor("biasC", [128, 1], f32, kind="ExternalInput").ap()
    woB = nc.dram_tensor("woB", [64, D], bf16, kind="ExternalInput").ap()
    yP = nc.dram_tensor("yP", [128, (s // 512) * 4096], bf16, kind="ExternalOutput").ap()

    with tile.TileContext(nc) as tc, ExitStack() as ctx:
        consts = ctx.enter_context(tc.tile_pool(name="consts", bufs=1))
        persist = ctx.enter_context(tc.tile_pool(name="persist", bufs=1))
        xt_pool = ctx.enter_context(tc.tile_pool(name="xt", bufs=3))
        wk_pool = ctx.enter_context(tc.tile_pool(name="work", bufs=6))
        probs_pool = ctx.enter_context(tc.tile_pool(name="probs", bufs=6))
        attn_pool = ctx.enter_context(tc.tile_pool(name="attn", bufs=3))
        yo_pool = ctx.enter_context(tc.tile_pool(name="yo", bufs=2))
        psum = ctx.enter_context(tc.tile_pool(name="psum", bufs=2, space="PSUM"))

        # ---- constants, spread across the three DMA queues.  gpsimd gets
        # everything phase1(0)'s q-pass touches (wq + x dc 0..3); sync gets
        # x dc 4..7; scalar gets the rest of phase1(0)'s needs.
        c_wq = consts.tile([128, D], bf16)
        c_cs = consts.tile([128, 2 * s], bf16)
        xts0 = xt_pool.tile([128, nd * 512], bf16)
        for dc in range(nd):
            if dc < 4:
                nc.gpsimd.dma_start(
                    c_wq[:, 128 * dc : 128 * (dc + 1)],
                    wqT[:, 128 * dc : 128 * (dc + 1)],
                )
                nc.gpsimd.dma_start(
                    xts0[:, 512 * dc : 512 * (dc + 1)],
                    xP[:, 512 * dc : 512 * (dc + 1)],
                )
            else:
                nc.sync.dma_start(
                    xts0[:, 512 * dc : 512 * (dc + 1)],
                    xP[:, 512 * dc : 512 * (dc + 1)],
                )
        nc.gpsimd.dma_start(c_wq[:, 512:1024], wqT[:, 512:1024])
        nc.sync.dma_start(c_cs[:, 0:1024], csP[:, 0:1024])
        c_P = consts.tile([128, 128], bf16)
        nc.sync.dma_start(c_P[:], P128[:])
        c_wk = consts.tile([128, D], bf16)
        nc.sync.dma_start(c_wk[:], wkT[:])
        c_wv = consts.tile([128, D], bf16)
        nc.sync.dma_start(c_wv[:], wvT[:])
        c_I = consts.tile([128, 128], bf16)
        nc.sync.dma_start(c_I[:], I128[:])
        c_one2 = consts.tile([128, 130], bf16)  # cols 0:2 ones, 2:130 zeros
        nc.gpsimd.dma_start(c_one2[:], ones2[:])
        c_ones64 = consts.tile([128, 64], f32)
        nc.gpsimd.dma_start(c_ones64[:], ones64[:])
        c_bias = consts.tile([128, 1], f32)
        nc.gpsimd.dma_start(c_bias[:], biasC[:])
        # maskg (2MB) and wo are not needed until ~15us in; they go on the
        # scalar queue after the early blocks so they don't block x/cs.
        c_maskg = consts.tile([128, 128 * max(n_gen, 1)], f32)
        c_wo = consts.tile([128, D], bf16)

        c_woB = consts.tile([64, D], bf16)

        def late_consts():
            nc.scalar.dma_start(c_maskg[:], maskg[:])
            nc.scalar.dma_start(c_wo[:], woT[:])
            nc.scalar.dma_start(c_woB[:], woB[:])

        # ---- persistent activations ----
        # qf8/kf8: [32, 4S] fp8.  col t*S+c holds, for partition p:
        #   t=0: head A rope'd dim p      at position c
        #   t=1: head A rope'd dim 32+p
        #   t=2: head B rope'd dim p
        #   t=3: head B rope'd dim 32+p
        v_sb = persist.tile([128, nb * 130], bf16)  # per sk-chunk: [A 64|1|B 64|1]
        if SCORES_MODE == "fp8":
            qf8 = persist.tile([32, 4 * s], fp8)
            kf8 = persist.tile([32, 4 * s], fp8)
            qq = qf8[:].rearrange("p (t c) -> p t c", c=s)
            kk = kf8[:].rearrange("p (t c) -> p t c", c=s)
        else:
            qT2 = persist.tile([128, s], bf16)
            kT2 = persist.tile([128, s], bf16)
        stg_cur = {}  # live fp8 staging tile per tensor ("q"/"k")
        xts_tiles = {}

        def load_xts(sb):
            """Prefetch x block sb + its cos/sin chunk (emitted ~2 blocks
            ahead so the 1MB transfer hides under compute)."""
            if sb <= 0 or sb >= NJ or sb in xts_tiles:
                return
            ceng = nc.scalar if sb < 2 else nc.sync
            ceng.dma_start(c_cs[:, 1024 * sb : 1024 * (sb + 1)],
                           csP[:, 1024 * sb : 1024 * (sb + 1)])
            t = xt_pool.tile([128, nd * 512], bf16, name="xtsp")
            nc.sync.dma_start(t[:], xP[:, nd * 512 * sb : nd * 512 * (sb + 1)])
            xts_tiles[sb] = t

        # ones columns of v_aug (cols 64 and 129 of each 130-wide chunk)
        ones_view = v_sb[:].rearrange("p (c w) -> p c w", w=130)[:, :, 64:130:65]
        (nc.gpsimd if POOL_MODE == "on" else nc.vector).tensor_copy(
            ones_view, c_one2[:, None, 0:2].broadcast_to([128, nb, 2])
        )

        def phase1(sb):
            """Projections + rope + v-transpose for rows [512*sb, 512*sb+512).
            q/k land (via rope) fp8 in a 2-block staging tile, flushed to
            qf8/kf8 with partition-remap DMAs each block pair."""
            csl = slice(1024 * sb, 1024 * sb + 512)  # cos chunk in c_cs
            snl = slice(1024 * sb + 512, 1024 * (sb + 1))  # sin chunk
            xts = xts_tiles.pop(sb) if sb else xts0
            so = 512 * (sb % 2)  # column offset in the 2-block staging tiles
            for cw, which in ((c_wq, "q"), (c_wk, "k")):
                ps = psum.tile([128, 512], f32, tag="proj", bufs=2)
                for dc in range(nd):
                    nc.tensor.matmul(
                        ps[:], cw[:, 128 * dc : 128 * (dc + 1)],
                        xts[:, 512 * dc : 512 * (dc + 1)],
                        start=dc == 0, stop=dc == nd - 1,
                    )
                raw = wk_pool.tile([128, 512], bf16, tag="rope")
                nc.vector.tensor_copy(raw[:], ps[:])
                psw = psum.tile([128, 512], f32, tag="proj", bufs=2)
                nc.tensor.matmul(psw[:], c_P[:], raw[:], start=True, stop=True)
                t1 = wk_pool.tile([128, 512], bf16, tag="rope")
                peng = nc.gpsimd if POOL_MODE == "on" else nc.vector
                peng.tensor_mul(t1[:], raw[:], c_cs[:, csl])
                t2 = wk_pool.tile([128, 512], bf16, tag="rope")
                nc.vector.tensor_mul(t2[:], psw[:], c_cs[:, snl])
                if SCORES_MODE != "fp8":
                    dstT = qT2 if which == "q" else kT2
                    nc.vector.tensor_add(
                        dstT[:, 512 * sb : 512 * (sb + 1)], t1[:], t2[:]
                    )
                    continue
                if sb % 2 == 0:
                    stg_cur[which] = wk_pool.tile(
                        [128, 1024], fp8, tag="stg" + which, bufs=2,
                        name="stg" + which,
                    )
                stg = stg_cur[which]
                nc.vector.tensor_add(stg[:, so : so + 512], t1[:], t2[:])
                # flush staged blocks to the [32, 4S] DoubleRow layout
                if sb % 2 == 1 or sb == NJ - 1:
                    w = so + 512
                    c0 = 512 * (sb - sb % 2)
                    dst = qq if which == "q" else kk
                    deng = nc.sync
                    for t in range(4):
                        deng.dma_start(
                            dst[:, t, c0 : c0 + w],
                            stg[32 * t : 32 * (t + 1), 0:w],
                        )
            # v pass: psum -> sbuf, then per-128 transpose into natural layout
            psv = psum.tile([128, 512], f32, tag="proj", bufs=2)
            for dc in range(nd):
                nc.tensor.matmul(
                    psv[:], c_wv[:, 128 * dc : 128 * (dc + 1)],
                    xts[:, 512 * dc : 512 * (dc + 1)],
                    start=dc == 0, stop=dc == nd - 1,
                )
            vtt = wk_pool.tile([128, 512], bf16, tag="vtt", bufs=2)
            nc.vector.tensor_copy(vtt[:], psv[:])
            for k4 in range(4):
                sc = 4 * sb + k4
                pst = psum.tile([128, 1024], bf16, tag="proj", bufs=2)
                nc.tensor.transpose(
                    pst[:, 0:128], vtt[:, 128 * k4 : 128 * (k4 + 1)], c_I[:]
                )
                vdst = v_sb[:].rearrange("p (c w) -> p c w", w=65)[
                    :, 2 * sc : 2 * sc + 2, 0:64
                ]
                psrc = pst[:, 0:128].rearrange("p (c w) -> p c w", w=64)
                nc.vector.tensor_copy(vdst, psrc)

        def attn(J, slots, tail=False, last=False):
            """Attention for q rows [512J, 512J+512); interleaves `slots`
            (previous J's output-projection emitters) between score chunks.
            Returns this J's outproj slots.  tail=True delays slot
            consumption further (the previous normalize has no phase1
            to hide behind)."""
            kept = [
                i
                for i in range(nb)
                if any(cls[i, 4 * J + u] != 1 for u in range(4))
            ]
            if not kept:
                while slots:
                    slots.pop(0)()
                return []
            pvA = psum.tile([128, 512], f32, tag="pv", bufs=2)
            pvB = psum.tile([128, 512], f32, tag="pv", bufs=2)
            pvq = []  # chunks whose pv matmuls are not yet emitted

            def emit_pv():
                i, pAB, ot = pvq.pop(0)
                first, last = i == kept[0], i == kept[-1]
                nc.tensor.matmul(
                    pvA[0:65, ot:512], v_sb[:, 130 * i : 130 * i + 65],
                    pAB[:, ot:512], start=first, stop=last,
                )
                nc.tensor.matmul(
                    pvB[0:65, ot:512], v_sb[:, 130 * i + 65 : 130 * i + 130],
                    pAB[:, 512 + ot : 1024], start=first, stop=last,
                )

            # delay slot consumption: at(J-1) is produced by a normalize
            # chain that runs a few us past its last pv, so out-projection
            # slots emitted in the first chunks would stall the in-order PE.
            if tail:
                skip = min(len(kept) - 1, 6)
            else:
                skip = 2 if len(kept) > 6 else (1 if len(kept) > 4 else 0)
            nslot = (
                max(1, -(-len(slots) // (len(kept) - skip))) if slots else 0
            )
            for n, i in enumerate(kept):
                subs = [int(cls[i, 4 * J + u]) for u in range(4)]
                fk = min(u for u in range(4) if subs[u] != 1)
                o = 128 * fk
                psAB = psum.tile([128, 1024], f32, tag="sc", bufs=2)
                # masked-prefix trim: q-cols < o are fully masked for this
                # chunk; skip them in the score and pv matmuls (pv then
                # never reads the garbage probs there).  The first kept
                # chunk is never trimmed so its start=True zeroes the full
                # pv region.
                ot = o if n > 0 else 0
                isl = slice(128 * i, 128 * (i + 1))
                jsl = slice(512 * J + ot, 512 * (J + 1))
                if SCORES_MODE == "fp8":
                    nc.tensor.matmul(
                        psAB[:, ot:512], kk[:, 0:2, isl], qq[:, 0:2, jsl],
                        start=True, stop=True, perf_mode=DR,
                    )
                    nc.tensor.matmul(
                        psAB[:, 512 + ot : 1024], kk[:, 2:4, isl],
                        qq[:, 2:4, jsl],
                        start=True, stop=True, perf_mode=DR,
                    )
                else:
                    nc.tensor.matmul(
                        psAB[:, ot:512], kT2[0:64, isl], qT2[0:64, jsl],
                        start=True, stop=True, tile_position=(0, 0),
                    )
                    nc.tensor.matmul(
                        psAB[:, 512 + ot : 1024], kT2[64:128, isl],
                        qT2[64:128, jsl],
                        start=True, stop=True, tile_position=(64, 0),
                    )
                if n >= skip:
                    for _ in range(nslot):
                        if slots:
                            slots.pop(0)()
                if len(pvq) >= 2:
                    emit_pv()
                for u, cu in enumerate(subs):
                    if cu == 2:
                        gi = GEN_INDEX[(i, 4 * J + u)]
                        # one strided add covers head A and head B blocks
                        pview = psAB[:].rearrange("p (h w) -> p h w", h=2)[
                            :, :, 128 * u : 128 * (u + 1)
                        ]
                        mview = c_maskg[:, None, 128 * gi : 128 * (gi + 1)]
                        nc.vector.tensor_add(
                            pview, pview, mview.broadcast_to([128, 2, 128])
                        )
                # one exp covers both heads; masked subs get zero-filled after
                pAB = probs_pool.tile([128, 1024], bf16)
                if ot == 0:
                    nc.scalar.activation(
                        pAB[:, 0:1024], psAB[:, 0:1024], Exp,
                        bias=c_bias[:, 0:1], scale=sc_scale,
                    )
                else:  # trimmed chunk: the strip between the heads was
                    # never written; exp each head's valid region alone
                    nc.scalar.activation(
                        pAB[:, ot:512], psAB[:, ot:512], Exp,
                        bias=c_bias[:, 0:1], scale=sc_scale,
                    )
                    nc.scalar.activation(
                        pAB[:, 512 + ot : 1024], psAB[:, 512 + ot : 1024],
                        Exp, bias=c_bias[:, 0:1], scale=sc_scale,
                    )
                for u, cu in enumerate(subs):
                    if cu == 1 and u > fk:
                        for off in (0, 512):
                            usl = slice(off + 128 * u, off + 128 * (u + 1))
                            (nc.gpsimd if POOL_MODE == "on" else nc.vector).tensor_copy(
                                pAB[:, usl], c_one2[:, 2:130]
                            )
                pvq.append((i, pAB, ot))
            while pvq:
                emit_pv()
            while slots:
                slots.pop(0)()
            # normalize:  attn = pv[0:64] * (1/denom row 64); head B first
            # (it needs an extra SBUF->SBUF partition-shift DMA into
            # at[64:128] so the output projection can contract both heads
            # in one K=128 matmul).  The reciprocal runs directly on
            # partition 64; a K=1 fp32 matmul broadcasts it to 64 rows.
            at = attn_pool.tile([128, 512], bf16)
            for hb, pv in ((1, pvB), (0, pvA)):
                # copy PSUM->SBUF immediately so the pv bank frees for the
                # next J before the reciprocal chain runs.
                pvf = wk_pool.tile([65, 512], f32, tag="den", bufs=2)
                nc.vector.tensor_copy(pvf[0:65, :], pv[0:65, :])
                if NORM_MODE == "v3":
                    d0 = wk_pool.tile([1, 512], f32, tag="d0", bufs=4)
                    nc.sync.dma_start(d0[:], pvf[64:65, :])
                if NORM_MODE == "pe":
                    r0 = wk_pool.tile([65, 512], f32, tag="d0", bufs=4)
                    s0 = wk_pool.tile([65, 512], f32, tag="d0", bufs=4)
                    nc.vector.reciprocal_approx_accurate(
                        r0[64:65, :], pvf[64:65, :], s0[64:65, :]
                    )
                    bc = psum.tile([64, 512], f32, tag="proj", bufs=2)
                    nc.tensor.matmul(
                        bc[:], c_ones64[64:65, 0:64], r0[64:65, :],
                        start=True, stop=True, tile_position=(64, 0),
                    )
                    bcin = bc[:]
                elif NORM_MODE == "direct":
                    r0 = wk_pool.tile([65, 512], f32, tag="d0", bufs=4)
                    s0 = wk_pool.tile([65, 512], f32, tag="d0", bufs=4)
                    nc.vector.reciprocal_approx_accurate(
                        r0[64:65, :], pvf[64:65, :], s0[64:65, :]
                    )
                    bcs = wk_pool.tile([64, 512], f32, tag="bcs", bufs=2)
                    nc.gpsimd.partition_broadcast(bcs[:], r0[64:65, :])
                    bcin = bcs[:]
                else:
                    r0 = wk_pool.tile([1, 512], f32, tag="d0", bufs=4)
                    s0 = wk_pool.tile([1, 512], f32, tag="d0", bufs=4)
                    nc.vector.reciprocal_approx_accurate(r0[:], d0[:], s0[:])
                    bcs = wk_pool.tile([64, 512], f32, tag="bcs", bufs=2)
                    nc.gpsimd.partition_broadcast(bcs[:], r0[:])
                    bcin = bcs[:]
                if hb:
                    atB = attn_pool.tile([64, 512], bf16)
                    nc.vector.tensor_mul(atB[:], pvf[0:64, :], bcin)
                    if not last:
                        nc.sync.dma_start(at[64:128, :], atB[:])
                else:
                    nc.vector.tensor_mul(at[0:64, :], pvf[0:64, :], bcin)
            # build outproj slot closures for this J (emitted during J+1)
            yo = yo_pool.tile([128, 4096], bf16)
            nleft = [8]

            def mk(scn, do2):
                def emit(use_alt=False):
                    ssl2 = slice(128 * scn, 128 * (scn + 1))
                    dsl = slice(512 * do2, 512 * (do2 + 1))
                    pso = psum.tile([128, 512], f32, tag="proj", bufs=2)
                    if last:
                        nc.tensor.matmul(
                            pso[:], at[0:64, ssl2], c_wo[0:64, dsl],
                            start=True, stop=False,
                        )
                        nc.tensor.matmul(
                            pso[:], atB[:, ssl2], c_woB[:, dsl],
                            start=False, stop=True,
                        )
                    else:
                        nc.tensor.matmul(
                            pso[:], at[:, ssl2], c_wo[:, dsl],
                            start=True, stop=True,
                        )
                    osl = slice(
                        1024 * scn + 512 * do2, 1024 * scn + 512 * do2 + 512
                    )
                    if use_alt:
                        nc.scalar.copy(yo[:, osl], pso[:])
                    else:
                        nc.vector.tensor_copy(yo[:, osl], pso[:])
                    nleft[0] -= 1
                    # slots emit scn-major, so the first half of yo is
                    # complete at nleft==4: drain it early
                    if nleft[0] == 4:
                        nc.sync.dma_start(
                            yP[:, 4096 * J : 4096 * J + 2048], yo[:, 0:2048]
                        )
                    elif nleft[0] == 0:
                        nc.sync.dma_start(
                            yP[:, 4096 * J + 2048 : 4096 * (J + 1)],
                            yo[:, 2048:4096],
                        )

                return emit

            return [mk(scn, do2) for scn in range(4) for do2 in range(2)]

        # attn(J) reads k/v blocks up to max(kept); interleaving phase1/attn
        # is only legal when that never exceeds the last-written block.
        # attn(J) is emitted after phase1(J+1) because even-numbered blocks
        # only flush to qf8/kf8 together with the following odd block.
        causal_ok = all(
            i <= 4 * J + 3
            for J in range(NJ)
            for i in range(nb)
            if any(cls[i, 4 * J + u] != 1 for u in range(4))
        )
        slots = []
        if causal_ok and NJ > 1 and SCORES_MODE == "fp8":
            # attn(0) is the smallest block: run it LAST so the pipeline
            # drains with the least unoverlappable softmax work.  fp8 mode
            # needs attn(J) after phase1(J+1) for the block-pair flush.
            phase1(0)
            phase1(1)
            late_consts()
            for J in range(1, NJ):
                if J + 1 < NJ:
                    phase1(J + 1)
                slots = attn(J, slots)
            slots = attn(0, slots)
        elif causal_ok and NJ > 1:
            # v3 ordering: attn(sb) right after phase1(sb); attn(0) last.
            load_xts(1)
            phase1(0)
            load_xts(2)
            for sb in range(1, NJ):
                phase1(sb)
                if sb == 1:
                    late_consts()
                load_xts(sb + 2)
                slots = attn(sb, slots)
            slots = attn(0, slots)
        else:
            for sb in range(NJ):
                load_xts(sb)
                phase1(sb)
                if sb == min(1, NJ - 1):
                    late_consts()
                load_xts(sb + 1)
            for J in range(NJ):
                slots = attn(J, slots)
        for n, emit in enumerate(slots):
            emit(use_alt=n % 2 == 1)

    nc.compile()
    return nc


GEN_INDEX = {}


def host_prep(x, freqs_cos, freqs_sin, mask, wq, wk, wv, wo):
    """Build per-core input maps + mask classification.  Returns
    (in_maps, cls, n_gen, neg_bias, sc_scale)."""
    global GEN_INDEX
    import ml_dtypes

    bf16 = ml_dtypes.bfloat16
    s = x.shape[1]
    nd, NJ_ = D // 128, s // 512
    x2 = np.ascontiguousarray(x.reshape(s, D))
    xT = np.ascontiguousarray(x2.T).astype(bf16)
    # packed layout: xP[p, sb*nd*512 + dc*512 + sl] = xT[128*dc+p, 512*sb+sl]
    xP = np.ascontiguousarray(
        xT.reshape(nd, 128, NJ_, 512).transpose(1, 2, 0, 3).reshape(128, nd * s)
    )

    # rope tables in T layout (same for q and k; scales folded into wq/wk)
    p = np.arange(128)
    j = (p % HD) // 2  # freq index per partition row
    cosT = np.ascontiguousarray(freqs_cos.T[j, :]).astype(bf16)  # [128, s]
    sinT = freqs_sin.T[j, :]
    sign = np.where(p % 2 == 0, -1.0, 1.0).astype(np.float32)
    sinTS = np.ascontiguousarray(sinT * sign[:, None]).astype(bf16)
    # cos|sin interleaved per 512-chunk: csP[p, sb*1024 + {0:512 cos, 512:1024 sin}]
    csP = np.ascontiguousarray(
        np.concatenate(
            [cosT.reshape(128, NJ_, 512), sinTS.reshape(128, NJ_, 512)], axis=2
        ).reshape(128, 2 * s)
    )

    # swap-adjacent permutation and identity
    P = np.zeros((128, 128), np.float32)
    P[np.arange(128) ^ 1, np.arange(128)] = 1.0
    I = np.eye(128, dtype=np.float32)

    # softmax shift + fp8 scale factors, from exact host projections
    qf = x2 @ wq.T
    kf = x2 @ wk.T
    qh = _rope_np(qf, freqs_cos, freqs_sin)
    kh = _rope_np(kf, freqs_cos, freqs_sin)
    m = 0.0
    for h in range(H):
        qs = qh[:, HD * h : HD * (h + 1)]
        ks = kh[:, HD * h : HD * (h + 1)]
        m = max(m, float(np.abs(qs @ ks.T).max()) / 8.0)
    neg_bias = max(0.0, m - 60.0)
    sQ = 224.0 / max(float(np.abs(qh).max()) * 0.125, 1e-30)
    sK = 224.0 / max(float(np.abs(kh).max()), 1e-30)
    sc_scale = 1.0 / (sQ * sK)

    # mask classification + general block packing (mask values pre-scaled
    # by sQ*sK so the activation scale recovers them exactly)
    maskT = np.ascontiguousarray(mask.T).astype(np.float32)
    cls = classify_mask(maskT, s)
    GEN_INDEX = {}
    gen_blocks = []
    nbk = s // 128
    for i in range(nbk):
        for jj in range(nbk):
            if cls[i, jj] == 2:
                GEN_INDEX[(i, jj)] = len(gen_blocks)
                gen_blocks.append(
                    maskT[128 * i : 128 * (i + 1), 128 * jj : 128 * (jj + 1)]
                    * (sQ * sK)
                )
    n_gen = len(gen_blocks)
    if n_gen:
        maskg = np.ascontiguousarray(np.concatenate(gen_blocks, axis=1))
        maskg = np.maximum(maskg, -3e38)
    else:
        maskg = np.zeros((128, 128), np.float32)

    def pack_w(w_slice):  # [EC, D] -> [128, D] chunked-transpose layout
        t = np.ascontiguousarray(w_slice.T)  # [D, EC=128]
        return np.ascontiguousarray(
            t.reshape(D // 128, 128, 128).transpose(1, 0, 2).reshape(128, D)
        ).astype(bf16)

    ones130 = np.zeros((128, 130), np.float32)
    ones130[:, 0:2] = 1.0
    ones64 = np.ones((128, 64), np.float32)
    biasC = np.full((128, 1), -neg_bias, np.float32)
    in_maps = []
    for c in range(NCORES):
        R = slice(EC * c, EC * (c + 1))
        woc = wo[:, R]  # [D, 128]
        woT_pack = np.ascontiguousarray(woc.T).astype(bf16)  # [128, D]
        woB_pack = np.ascontiguousarray(woT_pack[64:128])  # [64, D]
        in_maps.append(
            {
                "xP": xP,
                "csP": csP,
                "wqT": pack_w(wq[R] * (0.125 * sQ)),
                "wkT": pack_w(wk[R] * sK),
                "wvT": pack_w(wv[R]),
                "woT": woT_pack,
                "P128": P.astype(bf16),
                "I128": I.astype(bf16),
                "maskg": maskg,
                "ones2": ones130.astype(bf16),
                "ones64": ones64,
                "biasC": biasC,
                "woB": woB_pack,
            }
        )
    return in_maps, cls, n_gen, neg_bias, sc_scale


def _rope_np(t, cos, sin):
    s = t.shape[0]
    tr = t.reshape(s, H, HD // 2, 2)
    te, to = tr[..., 0], tr[..., 1]
    c = cos[:, None, :]
    sn = sin[:, None, :]
    oe = te * c - to * sn
    oo = te * sn + to * c
    return np.stack([oe, oo], axis=-1).reshape(s, H * HD)


def kernel(**inputs):
    from concourse.bass_utils import run_bass_kernel_spmd

    x = np.asarray(inputs["x"], np.float32)
    in_maps, cls, n_gen, neg_bias, sc_scale = host_prep(
        x,
        np.asarray(inputs["freqs_cos"], np.float32),
        np.asarray(inputs["freqs_sin"], np.float32),
        np.asarray(inputs["mask"], np.float32),
        np.asarray(inputs["wq"], np.float32),
        np.asarray(inputs["wk"], np.float32),
        np.asarray(inputs["wv"], np.float32),
        np.asarray(inputs["wo"], np.float32),
    )
    key = (x.shape[1], cls.tobytes(), n_gen, float(neg_bias), float(sc_scale))
    if key not in _PROGRAM_CACHE:
        _PROGRAM_CACHE[key] = build_program(
            x.shape[1], cls, n_gen, neg_bias, sc_scale
        )
    nc = _PROGRAM_CACHE[key]
    res = run_bass_kernel_spmd(nc, in_maps, core_ids=list(range(NCORES)))
    s = x.shape[1]
    yP = np.zeros((128, (s // 512) * 4096), np.float32)
    for c in range(NCORES):
        yP += np.asarray(res.results[c]["yP"], np.float32)
    # yP[p, J*4096 + scn*1024 + d] = y[512J + 128*scn + p, d]
    y = yP.reshape(128, s // 512, 4, D).transpose(1, 2, 0, 3).reshape(s, D)
    return np.ascontiguousarray(y).reshape(x.shape)
